# revision 1
# baseline (speedup 1.0000x reference)
import sys

sys.path.insert(0, "/opt/trn_rl_repo")

import numpy as np
import ml_dtypes

import concourse.bass as bass
import concourse.tile as tile
from concourse import bacc, mybir
from concourse.bass_utils import run_bass_kernel_spmd

BF16 = mybir.dt.bfloat16
F32 = mybir.dt.float32
U32 = mybir.dt.uint32

N_CORES = 8
P = 128
WAY, SHOT, SEQ, TSS = 5, 5, 10, 2
T = 45                      # C(10,2) tuple pairs
NQ = 75                     # global queries
NQL = 10                    # queries per core (core 7: 5 real + 5 dup)
QROWS = NQL * T             # 450
QPAD = 512                  # 4 x 128
QT = 4                      # qrow tiles
SROWS = WAY * SHOT * T      # 1125
SPAD = 1152                 # 9 x 128
ST = 9                      # srow tiles
CB = SHOT * T               # 225 support cols per class
D = 2048                    # out dim
KT = 16                     # feat tiles (128 each)
FQ = NQL * SEQ              # 100 query frames
FS = WAY * SHOT * SEQ       # 250 support frames
F = FQ + FS                 # 350
NSPLITS = [(0, 384), (384, 768), (768, 1152)]
BIG = 1.0e18

# run lengths of PAIRS grouped by first index i: i pairs with j=i+1..9
RUNS = [(i, 9 - i) for i in range(SEQ - 1)]
TSTART = [0]
for _, r in RUNS[:-1]:
    TSTART.append(TSTART[-1] + r)


def _build():
    nc = bacc.Bacc(None, num_devices=N_CORES)

    # ---- I/O ----
    wt = nc.declare_dram_parameter("wt", [KT, P, 2 * KT * P], BF16, isOutput=False)
    xt = nc.declare_dram_parameter("xt", [P, KT * F], BF16, isOutput=False)
    bias = nc.declare_dram_parameter("bias", [P, KT], F32, isOutput=False)
    invb = nc.declare_dram_parameter("invb", [P, QT], F32, isOutput=False)
    gmat = nc.declare_dram_parameter("gmat", [P, QT * 16], F32, isOutput=False)
    out_v = nc.declare_dram_parameter("out_v", [16, 16], F32, isOutput=True)
    out_ms = nc.declare_dram_parameter("out_ms", [WAY, 1], F32, isOutput=True)

    # internal DRAM
    ss2_c = [nc.dram_tensor(f"ss2_{c}", [CB, SPAD], F32) for c in range(WAY)]
    rec_in = nc.dram_tensor("rec_in", [WAY, SPAD], F32)
    rec_out = nc.dram_tensor("rec_out", [WAY, SPAD], F32, addr_space="Shared")

    with tile.TileContext(nc) as tc:
        with tc.tile_pool(name="persist", bufs=1) as pp:
            # ---- persistent SBUF ----
            sb_xt = pp.tile([P, KT * F], BF16)
            nc.sync.dma_start(out=sb_xt[:], in_=xt[:])
            sb_bias = pp.tile([P, KT], F32)
            nc.sync.dma_start(out=sb_bias[:], in_=bias[:])
            sb_invb = pp.tile([P, QT], F32)
            nc.sync.dma_start(out=sb_invb[:], in_=invb[:])
            sb_gmat = pp.tile([P, QT * 16], F32)
            nc.sync.dma_start(out=sb_gmat[:], in_=gmat[:])

            ES = pp.tile([P, KT, SPAD], BF16)
            EQ = pp.tile([P, KT, QPAD], BF16)
            D2 = pp.tile([P, QT, SPAD], F32)

            ones_c = pp.tile([P, 1], BF16)
            nc.vector.memset(ones_c[:], 1.0)
            ones_m = pp.tile([P, P], BF16)
            nc.vector.memset(ones_m[:], 1.0)

            qnc = pp.tile([P, QT], F32)
            snc = pp.tile([P, ST], F32)
            snb = pp.tile([P, SPAD], F32)

            import os
            bisect = int(os.environ.get("KBISECT", "0"))
            _phase1(nc, tc, sb_xt, sb_bias, wt, ES, EQ, ones_c, ones_m,
                    qnc, snc, snb)
            if bisect == 1:
                dbg = pp.tile([16, 16], F32)
                nc.vector.tensor_copy(dbg[:], EQ[:16, 0, :16])
                nc.sync.dma_start(out=out_v[:], in_=dbg[:])
            else:
                _phase2(nc, tc, ES, EQ, D2, qnc, snc, snb, ss2_c)
                if bisect == 2:
                    dbg = pp.tile([16, 16], F32)
                    nc.vector.tensor_copy(dbg[:], D2[:16, 0, :16])
                    nc.sync.dma_start(out=out_v[:], in_=dbg[:])
                else:
                    _phase3(nc, tc, D2, sb_invb, sb_gmat, ones_c, ss2_c,
                            rec_in, rec_out, out_v, out_ms, bisect)

    nc.compile()
    return nc


def _phase1(nc, tc, sb_xt, sb_bias, wt, ES, EQ, ones_c, ones_m, qnc, snc, snb):
    with (
        tc.tile_pool(name="wpool", bufs=2) as wp,
        tc.tile_pool(name="work", bufs=2) as wk,
        tc.tile_pool(name="ps_ab", bufs=1, space="PSUM") as ps_ab,
        tc.tile_pool(name="ps_norm", bufs=1, space="PSUM") as ps_norm,
    ):
            nc.vector.memset(qnc[:], 0.0)
            nc.vector.memset(snc[:], 0.0)
            nc.vector.memset(snb[:], 0.0)

            # ---- phase 1: embed (A/B matmuls) + tuple expand ----
            for m in range(KT):
                wm = wp.tile([P, 2 * KT * P], BF16, tag="wm")
                nc.gpsimd.dma_start(out=wm[:], in_=wt[m])
                psA = ps_ab.tile([P, F], F32, tag="psA")
                psB = ps_ab.tile([P, F], F32, tag="psB")
                for k in range(KT):
                    nc.tensor.matmul(
                        out=psA[:], lhsT=wm[:, k * P:(k + 1) * P], rhs=sb_xt[:, k * F:(k + 1) * F],
                        start=(k == 0), stop=(k == KT - 1))
                for k in range(KT):
                    nc.tensor.matmul(
                        out=psB[:], lhsT=wm[:, (KT + k) * P:(KT + k + 1) * P], rhs=sb_xt[:, k * F:(k + 1) * F],
                        start=(k == 0), stop=(k == KT - 1))

                sbB = wk.tile([P, F], F32, tag="sbB")
                nc.vector.tensor_copy(out=sbB[:], in_=psB[:])
                tmpq = wk.tile([P, QROWS], F32, tag="tmpq")
                tmps = wk.tile([P, SROWS], F32, tag="tmps")
                for (i, run), ts0 in zip(RUNS, TSTART):
                    # queries: rows n*45+t <- A[:, n*10+i] + B[:, n*10+j]
                    o = tmpq[:].rearrange("p (n t) -> p n t", n=NQL)[:, :, ts0:ts0 + run]
                    a = bass.AP(psA.tensor, psA[:].offset + i, [psA[:].ap[0], [SEQ, NQL], [0, run]])
                    b = bass.AP(sbB.tensor, sbB[:].offset + i + 1, [sbB[:].ap[0], [SEQ, NQL], [1, run]])
                    nc.vector.tensor_add(out=o, in0=a, in1=b)
                    # support frames start at col FQ
                    o = tmps[:].rearrange("p (n t) -> p n t", n=WAY * SHOT)[:, :, ts0:ts0 + run]
                    a = bass.AP(psA.tensor, psA[:].offset + FQ + i, [psA[:].ap[0], [SEQ, WAY * SHOT], [0, run]])
                    b = bass.AP(sbB.tensor, sbB[:].offset + FQ + i + 1, [sbB[:].ap[0], [SEQ, WAY * SHOT], [1, run]])
                    nc.vector.tensor_add(out=o, in0=a, in1=b)

                nc.scalar.activation(
                    out=EQ[:, m, :QROWS], in_=tmpq[:],
                    func=mybir.ActivationFunctionType.Relu,
                    bias=sb_bias[:, m:m + 1])
                nc.vector.memset(EQ[:, m, QROWS:], 0.0)
                nc.scalar.activation(
                    out=ES[:, m, :SROWS], in_=tmps[:],
                    func=mybir.ActivationFunctionType.Relu,
                    bias=sb_bias[:, m:m + 1])
                nc.vector.memset(ES[:, m, SROWS:], 0.0)

                # squares (bf16) for norms
                eq2 = wk.tile([P, QPAD], BF16, tag="eq2")
                es2 = wk.tile([P, SPAD], BF16, tag="es2")
                nc.vector.tensor_mul(out=eq2[:], in0=EQ[:, m, :], in1=EQ[:, m, :])
                nc.vector.tensor_mul(out=es2[:], in0=ES[:, m, :], in1=ES[:, m, :])
                # norm partial sums for this feat tile, accumulated in SBUF
                qnp = ps_norm.tile([P, QT], F32, tag="qnp")
                for mq in range(QT):
                    nc.tensor.matmul(
                        out=qnp[:, mq:mq + 1], lhsT=eq2[:, mq * P:(mq + 1) * P],
                        rhs=ones_c[:], start=True, stop=True)
                nc.vector.tensor_add(out=qnc[:], in0=qnc[:], in1=qnp[:])
                snp = ps_norm.tile([P, ST], F32, tag="snp")
                for ms in range(ST):
                    nc.tensor.matmul(
                        out=snp[:, ms:ms + 1], lhsT=es2[:, ms * P:(ms + 1) * P],
                        rhs=ones_c[:], start=True, stop=True)
                nc.vector.tensor_add(out=snc[:], in0=snc[:], in1=snp[:])
                # sn broadcast rows: all-ones lhsT -> every partition gets sn[s]
                for sp, (lo, hi) in enumerate(NSPLITS):
                    snbp = ps_norm.tile([P, 384], F32, tag=f"snbp{sp}")
                    nc.tensor.matmul(
                        out=snbp[:], lhsT=ones_m[:], rhs=es2[:, lo:hi],
                        start=True, stop=True)
                    nc.vector.tensor_add(out=snb[:, lo:hi], in0=snb[:, lo:hi], in1=snbp[:])



def _phase2(nc, tc, ES, EQ, D2, qnc, snc, snb, ss2_c):
        # ---- phase 2: grams -> squared distances ----
        with (
            tc.tile_pool(name="ps_gram", bufs=4, space="PSUM") as ps_g,
            tc.tile_pool(name="gwork", bufs=3) as gw,
        ):
            # query-to-support
            for mq in range(QT):
                for lo, hi in NSPLITS:
                    pg = ps_g.tile([P, 384], F32, tag="pg")
                    for k in range(KT):
                        nc.tensor.matmul(
                            out=pg[:], lhsT=EQ[:, k, mq * P:(mq + 1) * P],
                            rhs=ES[:, k, lo:hi], start=(k == 0), stop=(k == KT - 1))
                    tmp = gw.tile([P, 384], F32, tag="gtmp")
                    nc.vector.scalar_tensor_tensor(
                        out=tmp[:], in0=pg[:], scalar=-2.0, in1=snb[:, lo:hi],
                        op0=mybir.AluOpType.mult, op1=mybir.AluOpType.add)
                    nc.scalar.activation(
                        out=D2[:, mq, lo:hi], in_=tmp[:],
                        func=mybir.ActivationFunctionType.Relu,
                        bias=qnc[:, mq:mq + 1])
            # support-to-support -> DRAM (per-class row blocks)
            for ms in range(ST):
                ss2t = gw.tile([P, SPAD], F32, tag="ss2t")
                for lo, hi in NSPLITS:
                    pg = ps_g.tile([P, 384], F32, tag="pg")
                    for k in range(KT):
                        nc.tensor.matmul(
                            out=pg[:], lhsT=ES[:, k, ms * P:(ms + 1) * P],
                            rhs=ES[:, k, lo:hi], start=(k == 0), stop=(k == KT - 1))
                    tmp = gw.tile([P, 384], F32, tag="gtmp")
                    nc.vector.scalar_tensor_tensor(
                        out=tmp[:], in0=pg[:], scalar=-2.0, in1=snb[:, lo:hi],
                        op0=mybir.AluOpType.mult, op1=mybir.AluOpType.add)
                    nc.scalar.activation(
                        out=ss2t[:, lo:hi], in_=tmp[:],
                        func=mybir.ActivationFunctionType.Relu,
                        bias=snc[:, ms:ms + 1])
                # scatter the 128 rows into per-class tensors
                r0 = ms * P
                r1 = min(r0 + P, SROWS)
                r = r0
                while r < r1:
                    c = r // CB
                    re = min(r1, (c + 1) * CB)
                    nc.sync.dma_start(
                        out=ss2_c[c][r - c * CB:re - c * CB, :],
                        in_=ss2t[r - r0:re - r0, :])
                    r = re

def _phase3(nc, tc, D2, sb_invb, sb_gmat, ones_c, ss2_c,
            rec_in, rec_out, out_v, out_ms, bisect=0):
        # ---- phase 3: per-class reductions, gather, record ----
        with (
            tc.tile_pool(name="red", bufs=1) as rp,
            tc.tile_pool(name="gath", bufs=2) as gp,
            tc.tile_pool(name="ps3", bufs=2, space="PSUM") as ps3,
            tc.tile_pool(name="dram3", bufs=1, space="DRAM") as dr3,
        ):
            V = rp.tile([P, QT, 16], F32)
            nc.vector.memset(V[:], 0.0)
            ave2 = rp.tile([P, QT, WAY], F32)
            idx = rp.tile([P, QT, WAY], U32)
            m8 = rp.tile([P, 8], F32)
            i8 = rp.tile([P, 8], U32)
            dcall = rp.tile([P, QT, WAY, SHOT], F32)

            for c in range(WAY):
                for mq in range(QT):
                    blk = D2[:, mq, c * CB:(c + 1) * CB]
                    nc.vector.max(m8[:], blk)
                    nc.vector.max_index(i8[:], m8[:], blk)
                    nc.vector.tensor_copy(idx[:, mq, c:c + 1], i8[:, 0:1])
                    nc.vector.tensor_copy(V[:, mq, c:c + 1], m8[:, 0:1])
                    # dist_class: max over i of col i*5+j
                    g = blk.rearrange("p (i j) -> p j i", j=SHOT)
                    nc.vector.reduce_max(dcall[:, mq, c, :], g,
                                         axis=mybir.AxisListType.X)
            # batched sqrt / mean / square+invalid-bias per qrow tile
            for mq in range(QT):
                nc.scalar.sqrt(out=V[:, mq, :WAY], in_=V[:, mq, :WAY])
                nc.scalar.sqrt(out=dcall[:, mq], in_=dcall[:, mq])
                nc.vector.reduce_sum(ave2[:, mq, :], dcall[:, mq],
                                     axis=mybir.AxisListType.X)
                nc.scalar.activation(
                    out=ave2[:, mq, :], in_=ave2[:, mq, :],
                    func=mybir.ActivationFunctionType.Square,
                    bias=sb_invb[:, mq:mq + 1], scale=1.0 / SHOT)

            if bisect == 3:
                dbg = rp.tile([16, 16], F32)
                nc.vector.memset(dbg[:], 0.0)
                nc.vector.tensor_copy(dbg[:, :WAY], ave2[:16, 0, :])
                nc.sync.dma_start(out=out_v[:], in_=dbg[:])
                return

            rec_sb = rp.tile([8, SPAD], F32)
            for c in range(WAY):
                cmp = gp.tile([P, QT, SPAD], BF16, tag="cmp")
                for mq in range(QT):
                    ssg = gp.tile([P, SPAD], F32, tag="ssg")
                    nc.gpsimd.indirect_dma_start(
                        out=ssg[:], out_offset=None,
                        in_=ss2_c[c][:],
                        in_offset=bass.IndirectOffsetOnAxis(ap=idx[:, mq, c:c + 1], axis=0))
                    nc.vector.tensor_scalar(
                        out=cmp[:, mq, :], in0=ssg[:],
                        scalar1=ave2[:, mq, c:c + 1], scalar2=None,
                        op0=mybir.AluOpType.is_gt)
                    # exclude own-class columns from the record counts
                    nc.vector.memset(cmp[:, mq, c * CB:(c + 1) * CB], 0.0)
                for lo, hi in NSPLITS:
                    pr = ps3.tile([1, 384], F32, tag="pr")
                    for mq in range(QT):
                        nc.tensor.matmul(
                            out=pr[:], lhsT=ones_c[:], rhs=cmp[:, mq, lo:hi],
                            start=(mq == 0), stop=(mq == QT - 1))
                    prs = gp.tile([1, 384], F32, tag="prs")
                    nc.scalar.copy(out=prs[:], in_=pr[:])
                    nc.sync.dma_start(out=rec_sb[c:c + 1, lo:hi], in_=prs[:])

            if bisect == 4:
                dbg = rp.tile([16, 16], F32)
                nc.vector.memset(dbg[:], 0.0)
                nc.vector.tensor_copy(dbg[:WAY, :], rec_sb[:WAY, :16])
                nc.sync.dma_start(out=out_v[:], in_=dbg[:])
                return

            # global sum of record counts
            ci = dr3.tile([WAY, SPAD], F32)
            nc.sync.dma_start(out=ci[:], in_=rec_sb[:WAY, :])
            co = dr3.tile([WAY, SPAD], F32, addr_space="Shared")
            nc.gpsimd.collective_compute(
                "AllReduce", mybir.AluOpType.add,
                replica_groups=[list(range(N_CORES))],
                ins=[ci[:]], outs=[co[:]])
            recg = rp.tile([8, SPAD], F32)
            nc.sync.dma_start(out=recg[:WAY, :], in_=co[:])
            if bisect == 5:
                dbg = rp.tile([16, 16], F32)
                nc.vector.memset(dbg[:], 0.0)
                nc.vector.tensor_copy(dbg[:WAY, :], recg[:WAY, :16])
                nc.sync.dma_start(out=out_v[:], in_=dbg[:])
                return

            # thr per (class, block); mask; msum
            rsum = rp.tile([8, WAY], F32)
            nzc = rp.tile([8, SPAD], F32)
            nz = rp.tile([8, WAY], F32)
            thr = rp.tile([8, WAY], F32)
            g = recg[:WAY, :SROWS].rearrange("p (b s) -> p b s", b=WAY)
            nc.vector.reduce_sum(rsum[:WAY, :], g, axis=mybir.AxisListType.X)
            nc.vector.tensor_scalar(
                out=nzc[:WAY, :SROWS], in0=recg[:WAY, :SROWS],
                scalar1=0.5, scalar2=None, op0=mybir.AluOpType.is_gt)
            nc.vector.reduce_sum(
                nz[:WAY, :], nzc[:WAY, :SROWS].rearrange("p (b s) -> p b s", b=WAY),
                axis=mybir.AxisListType.X)
            nc.vector.tensor_scalar(
                out=nz[:WAY, :], in0=nz[:WAY, :], scalar1=1.0, scalar2=None,
                op0=mybir.AluOpType.max)
            nc.vector.reciprocal(out=nz[:WAY, :], in_=nz[:WAY, :])
            nc.vector.tensor_mul(out=thr[:WAY, :], in0=rsum[:WAY, :], in1=nz[:WAY, :])

            mask = rp.tile([8, SPAD], F32)
            for b in range(WAY):
                nc.vector.tensor_scalar(
                    out=mask[:WAY, b * CB:(b + 1) * CB],
                    in0=recg[:WAY, b * CB:(b + 1) * CB],
                    scalar1=thr[:WAY, b:b + 1], scalar2=None,
                    op0=mybir.AluOpType.is_lt)
            msum = rp.tile([8, 1], F32)
            nc.vector.reduce_sum(msum[:WAY, :], mask[:WAY, :SROWS], axis=mybir.AxisListType.X)
            nc.sync.dma_start(out=out_ms[:], in_=msum[:WAY, :])
            if bisect == 6:
                dbg = rp.tile([16, 16], F32)
                nc.vector.memset(dbg[:], 0.0)
                nc.vector.tensor_copy(dbg[:WAY, :], mask[:WAY, :16])
                nc.sync.dma_start(out=out_v[:], in_=dbg[:])
                return

            # masked mean distances (uses sqrt distances)
            Ds = gp.tile([P, QT, SPAD], F32, tag="ds")
            for mq in range(QT):
                nc.scalar.sqrt(out=Ds[:, mq, :], in_=D2[:, mq, :])
            if bisect == 71:
                dbg = rp.tile([16, 16], F32)
                nc.vector.tensor_copy(dbg[:], Ds[:16, 0, :16])
                nc.sync.dma_start(out=out_v[:], in_=dbg[:])
                return
            scr = gp.tile([P, SROWS], F32, tag="scr")
            ones_r = rp.tile([1, P], F32)
            nc.vector.memset(ones_r[:], 1.0)
            for c in range(WAY):
                mrow = gp.tile([1, SROWS], F32, tag="mrow")
                nc.sync.dma_start(out=mrow[:], in_=mask[c:c + 1, :SROWS])
                mb = gp.tile([P, SROWS], F32, tag="mb")
                for sp, (lo, hi) in enumerate(NSPLITS):
                    w = min(hi, SROWS) - lo
                    pb = ps3.tile([P, 384], F32, tag="pb")
                    nc.tensor.matmul(out=pb[:, :w], lhsT=ones_r[:],
                                     rhs=mrow[:, lo:lo + w], start=True, stop=True)
                    nc.scalar.copy(out=mb[:, lo:lo + w], in_=pb[:, :w])
                if bisect != 72:
                    for mq in range(QT):
                        nc.vector.tensor_mul(out=scr[:], in0=Ds[:, mq, :SROWS], in1=mb[:])
                        nc.vector.reduce_sum(V[:, mq, WAY + c:WAY + c + 1], scr[:],
                                             axis=mybir.AxisListType.X)

            # group rows by query: out[q, col] = sum_r gmat[r, q] * V[r, col]
            if bisect in (7, 72):
                dbg = rp.tile([16, 16], F32)
                nc.vector.tensor_copy(dbg[:], V[:16, 0, :])
                nc.sync.dma_start(out=out_v[:], in_=dbg[:])
                return
            pv = ps3.tile([16, 16], F32, tag="pv", bufs=1)
            for t in range(QT):
                nc.tensor.matmul(
                    out=pv[:], lhsT=sb_gmat[:, t * 16:(t + 1) * 16], rhs=V[:, t, :],
                    start=(t == 0), stop=(t == QT - 1))
            vsb = rp.tile([16, 16], F32)
            nc.scalar.copy(out=vsb[:], in_=pv[:])
            nc.sync.dma_start(out=out_v[:], in_=vsb[:])


_CACHE = {}


def _get_nc():
    if "nc" not in _CACHE:
        _CACHE["nc"] = _build()
    return _CACHE["nc"]


def kernel(support_set, support_labels, queries, clsW_w, clsW_b):
    support_set = np.asarray(support_set, dtype=np.float32)
    queries = np.asarray(queries, dtype=np.float32)
    W = np.asarray(clsW_w, dtype=np.float32)
    b = np.asarray(clsW_b, dtype=np.float32)
    bf = ml_dtypes.bfloat16

    # weight tiles: wt[m, kp, h, ko, mf] = W[m*128+mf, h*2048+ko*128+kp]
    wt = np.ascontiguousarray(
        W.reshape(KT, P, 2, KT, P).transpose(0, 4, 2, 3, 1)).astype(bf)
    bias_h = np.ascontiguousarray(b.reshape(KT, P).T)

    # per-core frame matrices
    sup_frames = support_set.reshape(FS, D)
    in_maps = []
    rows = np.arange(QPAD)
    gm = np.zeros((QPAD, 16), np.float32)
    valid_all = rows < QROWS
    gm[valid_all, (rows // T)[valid_all].clip(0, 15)] = 1.0
    gmat_h = np.ascontiguousarray(gm.reshape(QT, P, 16).transpose(1, 0, 2))
    for k in range(N_CORES):
        qids = [(10 * k + j) % NQ for j in range(NQL)]
        qf = queries[qids].reshape(FQ, D)
        frames = np.concatenate([qf, sup_frames], axis=0)  # [350, 2048]
        xth = np.ascontiguousarray(
            frames.T.reshape(KT, P, F).transpose(1, 0, 2)).astype(bf)
        inv = np.where(rows < (QROWS if k < N_CORES - 1 else 5 * T), 0.0, BIG)
        inv = inv.astype(np.float32)
        invb_h = np.ascontiguousarray(inv.reshape(QT, P).T)
        in_maps.append({
            "wt": wt, "xt": xth, "bias": bias_h,
            "invb": invb_h, "gmat": gmat_h,
        })

    nc = _get_nc()
    _CACHE["last_in_maps"] = in_maps
    res = run_bass_kernel_spmd(nc, in_maps, list(range(N_CORES))).results

    msum = np.maximum(res[0]["out_ms"].reshape(WAY), 1.0)
    dist_max = np.zeros((NQ, WAY), np.float32)
    md_raw = np.zeros((NQ, WAY), np.float32)
    for k in range(N_CORES):
        v = res[k]["out_v"]
        for j in range(NQL):
            q = 10 * k + j
            if q >= NQ:
                break
            dist_max[q] = v[j, :WAY] / T
            md_raw[q] = v[j, WAY:2 * WAY]
    contrast = md_raw / (T * (WAY - 1) * msum[None, :])
    logits = dist_max / (contrast + dist_max)
    return dist_max, logits



# revision 14
# speedup vs baseline: 1.1898x; 1.1898x over previous
import sys

sys.path.insert(0, "/opt/trn_rl_repo")

import numpy as np
import ml_dtypes

import concourse.bass as bass
import concourse.tile as tile
from concourse import bacc, mybir
from concourse.bass_utils import run_bass_kernel_spmd

BF16 = mybir.dt.bfloat16
F32 = mybir.dt.float32
U32 = mybir.dt.uint32

N_CORES = 8
P = 128
WAY, SHOT, SEQ, TSS = 5, 5, 10, 2
T = 45                      # C(10,2) tuple pairs
NQ = 75                     # global queries
NQL = 10                    # queries per core (core 7: 5 real + 5 dup)
QROWS = NQL * T             # 450
QPAD = 512                  # 4 x 128
QT = 4                      # qrow tiles
SROWS = WAY * SHOT * T      # 1125
SPAD = 1152                 # 9 x 128
CB = SHOT * T               # 225 support cols per class
D = 2048                    # out dim
KT = 16                     # feat tiles (128 each)
FQ = NQL * SEQ              # 100 query frames
FS = WAY * SHOT * SEQ       # 250 support frames
F = FQ + FS                 # 350
NSPLITS = [(0, 384), (384, 768), (768, 1152)]
BIG = 1.0e18
XR = 180                    # s-s gram shard rows per core (4-sample window)
MG = 8                      # m-tiles per phase-1 group

# run lengths of PAIRS grouped by first index i: i pairs with j=i+1..9
RUNS = [(i, 9 - i) for i in range(SEQ - 1)]
TSTART = [0]
for _, r in RUNS[:-1]:
    TSTART.append(TSTART[-1] + r)


def _build():
    nc = bacc.Bacc(None, num_devices=N_CORES)

    # ---- I/O ----
    wt = nc.declare_dram_parameter("wt", [KT, P, 2 * KT * P], BF16, isOutput=False)
    xt = nc.declare_dram_parameter("xt", [P, KT * F], BF16, isOutput=False)
    bias = nc.declare_dram_parameter("bias", [P, KT], F32, isOutput=False)
    invb = nc.declare_dram_parameter("invb", [P, QT], F32, isOutput=False)
    gmat = nc.declare_dram_parameter("gmat", [P, QT * 16], F32, isOutput=False)
    out_v = nc.declare_dram_parameter("out_v", [16, 16], F32, isOutput=True)
    out_ms = nc.declare_dram_parameter("out_ms", [WAY, 1], F32, isOutput=True)

    # internal DRAM
    ci_ss = nc.dram_tensor("ci_ss", [XR, SPAD], BF16)
    ss_all = nc.dram_tensor("ss_all", [N_CORES * XR, SPAD], BF16, addr_space="Shared")
    rec_in = nc.dram_tensor("rec_in", [WAY, SPAD], F32)
    rec_out = nc.dram_tensor("rec_out", [WAY, SPAD], F32, addr_space="Shared")

    import os
    bisect = int(os.environ.get("KBISECT", "0"))

    with tile.TileContext(nc) as tc:
        with tc.tile_pool(name="persist", bufs=1) as pp:
            sb_bias = pp.tile([P, KT], F32)
            nc.sync.dma_start(out=sb_bias[:], in_=bias[:])
            sb_invb = pp.tile([P, QT], F32)
            nc.sync.dma_start(out=sb_invb[:], in_=invb[:])
            sb_gmat = pp.tile([P, QT * 16], F32)
            nc.sync.dma_start(out=sb_gmat[:], in_=gmat[:])

            EQ = pp.tile([P, KT, QPAD], BF16)    # relu'd query tuple rows
            ES = pp.tile([P, KT, SPAD], BF16)    # relu'd support tuple rows
            AQL = pp.tile([P, QPAD], BF16)       # aug lhsT for queries
            ASL = pp.tile([P, SPAD], BF16)       # aug lhsT for supports
            ASR = pp.tile([P, SPAD], BF16)       # aug rhs for supports
            D2 = pp.tile([P, QT, SPAD], BF16)    # squared q-s distances
            ones_c = pp.tile([P, 1], BF16)
            nc.vector.memset(ones_c[:], 1.0)

            _phase1(nc, tc, pp, xt, sb_bias, wt, EQ, ES,
                    AQL, ASL, ASR, ones_c)
            if bisect == 1:
                dbg = pp.tile([16, 16], F32)
                nc.vector.tensor_copy(dbg[:], EQ[:16, 0, :16])
                nc.sync.dma_start(out=out_v[:], in_=dbg[:])
            else:
                _phase2(nc, tc, pp, EQ, ES, AQL, ASL, ASR, D2, ci_ss, ss_all,
                        bisect, out_v)
                if bisect == 2:
                    dbg = pp.tile([16, 16], F32)
                    nc.vector.tensor_copy(dbg[:], D2[:16, 0, :16])
                    nc.sync.dma_start(out=out_v[:], in_=dbg[:])
                elif bisect != 25:
                    _phase3(nc, tc, pp, D2, sb_invb, sb_gmat, ss_all,
                            rec_in, rec_out, out_v, out_ms, bisect)

    nc.compile()
    return nc


def _phase1(nc, tc, pp, xt, sb_bias, wt, EQ, ES, AQL, ASL, ASR,
            ones_c):
    # zero pad columns and aug tiles up front
    nc.vector.memset(EQ[:, :, QROWS:], 0.0)
    nc.gpsimd.memset(ES[:, :, SROWS:], 0.0)
    nc.vector.memset(AQL[:], 0.0)
    nc.gpsimd.memset(ASL[:], 0.0)
    nc.vector.memset(ASR[:], 0.0)

    with (
        tc.tile_pool(name="ph1", bufs=1) as p1,
        tc.tile_pool(name="wpool", bufs=2) as wp,
        tc.tile_pool(name="sqp", bufs=2) as sqp,
        tc.tile_pool(name="ps_ab", bufs=2, space="PSUM") as ps_ab,
        tc.tile_pool(name="ps_norm", bufs=1, space="PSUM") as ps_n,
    ):
        sb_xt = p1.tile([P, KT * F], BF16)
        nc.sync.dma_start(out=sb_xt[:], in_=xt[:])
        EA = p1.tile([P, KT, F], BF16)       # A-half embed + bias, per frame
        EB = p1.tile([P, KT, F], BF16)       # B-half embed, per frame
        qn = ps_n.tile([1, QPAD], F32, tag="qn")
        sn = [ps_n.tile([1, 384], F32, tag=f"sn{j}", name=f"sn{j}")
              for j in range(3)]

        for g in range(KT // MG):
            for m in range(g * MG, (g + 1) * MG):
                wm = wp.tile([P, 2 * KT * P], BF16, tag="wm")
                nc.gpsimd.dma_start(out=wm[:], in_=wt[m])
                psA = ps_ab.tile([P, F], F32, tag="psA")
                psB = ps_ab.tile([P, F], F32, tag="psB")
                for k in range(KT):
                    nc.tensor.matmul(
                        out=psA[:], lhsT=wm[:, k * P:(k + 1) * P],
                        rhs=sb_xt[:, k * F:(k + 1) * F],
                        start=(k == 0), stop=(k == KT - 1))
                for k in range(KT):
                    nc.tensor.matmul(
                        out=psB[:], lhsT=wm[:, (KT + k) * P:(KT + k + 1) * P],
                        rhs=sb_xt[:, k * F:(k + 1) * F],
                        start=(k == 0), stop=(k == KT - 1))
                # evict with dtype cast; bias folded into A once per pair
                nc.scalar.activation(
                    out=EA[:, m, :], in_=psA[:],
                    func=mybir.ActivationFunctionType.Identity,
                    bias=sb_bias[:, m:m + 1])
                nc.scalar.copy(out=EB[:, m, :], in_=psB[:])

            # ---- batched tuple expansion for this m-group ----
            ms = slice(g * MG, (g + 1) * MG)
            EAq = EA[:, ms, :FQ].rearrange("p m (n s) -> p m n s", s=SEQ)
            EBq = EB[:, ms, :FQ].rearrange("p m (n s) -> p m n s", s=SEQ)
            EAs = EA[:, ms, FQ:].rearrange("p m (n s) -> p m n s", s=SEQ)
            EBs = EB[:, ms, FQ:].rearrange("p m (n s) -> p m n s", s=SEQ)
            EQv = EQ[:, ms, :QROWS].rearrange("p m (n t) -> p m n t", t=T)
            ESv = ES[:, ms, :SROWS].rearrange("p m (n t) -> p m n t", t=T)
            for (i, run), ts0 in zip(RUNS, TSTART):
                for tt in range(run):
                    t = ts0 + tt
                    j = i + 1 + tt
                    nc.vector.tensor_add(
                        out=EQv[:, :, :, t], in0=EAq[:, :, :, i],
                        in1=EBq[:, :, :, j])
                    nc.vector.tensor_add(
                        out=ESv[:, :, :, t], in0=EAs[:, :, :, i],
                        in1=EBs[:, :, :, j])
            # relu in place on the scalar engine
            nc.scalar.activation(
                out=ES[:, ms, :], in_=ES[:, ms, :],
                func=mybir.ActivationFunctionType.Relu)
            nc.scalar.activation(
                out=EQ[:, ms, :], in_=EQ[:, ms, :],
                func=mybir.ActivationFunctionType.Relu)
            # squares + norm row accumulation (over partitions via matmul)
            for m in range(g * MG, (g + 1) * MG):
                sq = sqp.tile([P, SPAD], BF16, tag="sq")
                nc.vector.tensor_mul(out=sq[:], in0=ES[:, m, :], in1=ES[:, m, :])
                sqq = sqp.tile([P, QPAD], BF16, tag="sqq")
                nc.vector.tensor_mul(out=sqq[:], in0=EQ[:, m, :], in1=EQ[:, m, :])
                for j, (lo, hi) in enumerate(NSPLITS):
                    nc.tensor.matmul(
                        out=sn[j][:], lhsT=ones_c[:], rhs=sq[:, lo:hi],
                        start=(m == 0), stop=(m == KT - 1))
                nc.tensor.matmul(
                    out=qn[:], lhsT=ones_c[:], rhs=sqq[:],
                    start=(m == 0), stop=(m == KT - 1))

        # ---- aug tiles (bf16x2 error-compensated norms) ----
        # ASR: p0=1, p1=1, p2=sn_hi, p3=sn_lo ; ASL: p0=-sn_hi/2, p1=-sn_lo/2,
        # p2=p3=-1/2 ; AQL: p0=-qn_hi/2, p1=-qn_lo/2, p2=p3=-1/2
        # compute-engine APs must start at partition 0, so rows for
        # partitions >= 1 are staged at p0 and DMA'd into place
        snf = p1.tile([1, SPAD], F32)
        for j, (lo, hi) in enumerate(NSPLITS):
            nc.scalar.copy(out=snf[:, lo:hi], in_=sn[j][:])
        snh_bf = p1.tile([1, SPAD], BF16)
        nc.scalar.copy(out=snh_bf[:], in_=snf[:])              # sn_hi
        snh = p1.tile([1, SPAD], F32)
        nc.scalar.copy(out=snh[:], in_=snh_bf[:])
        snl = p1.tile([1, SPAD], F32)
        nc.vector.tensor_sub(out=snl[:], in0=snf[:], in1=snh[:])
        snl_bf = p1.tile([1, SPAD], BF16)
        nc.vector.tensor_copy(out=snl_bf[:], in_=snl[:])       # sn_lo
        ashi = p1.tile([1, SPAD], BF16)
        nc.scalar.mul(out=ashi[:], in_=snh_bf[:], mul=-0.5)
        aslo = p1.tile([1, SPAD], BF16)
        nc.scalar.mul(out=aslo[:], in_=snl_bf[:], mul=-0.5)
        nc.vector.memset(ASR[0:2, :], 1.0)
        nc.sync.dma_start(out=ASR[2:3, :], in_=snh_bf[:])
        nc.sync.dma_start(out=ASR[3:4, :], in_=snl_bf[:])
        nc.vector.memset(ASL[0:4, :], -0.5)
        nc.sync.dma_start(out=ASL[0:1, :], in_=ashi[:])
        nc.sync.dma_start(out=ASL[1:2, :], in_=aslo[:])

        qnf = p1.tile([1, QPAD], F32)
        nc.scalar.copy(out=qnf[:], in_=qn[:])
        qh = p1.tile([1, QPAD], BF16)
        nc.scalar.mul(out=qh[:], in_=qnf[:], mul=-0.5)         # -qn/2 hi (bf16)
        qhf = p1.tile([1, QPAD], F32)
        nc.scalar.copy(out=qhf[:], in_=qh[:])
        qlf = p1.tile([1, QPAD], F32)
        # residual: (-qn/2) - hi
        nc.vector.scalar_tensor_tensor(
            out=qlf[:], in0=qnf[:], scalar=-0.5, in1=qhf[:],
            op0=mybir.AluOpType.mult, op1=mybir.AluOpType.subtract)
        ql = p1.tile([1, QPAD], BF16)
        nc.vector.tensor_copy(out=ql[:], in_=qlf[:])
        nc.vector.memset(AQL[0:4, :], -0.5)
        nc.sync.dma_start(out=AQL[0:1, :], in_=qh[:])
        nc.sync.dma_start(out=AQL[1:2, :], in_=ql[:])


def _phase2(nc, tc, pp, EQ, ES, AQL, ASL, ASR, D2, ci_ss, ss_all, bisect,
            out_v):
    with (
        tc.tile_pool(name="ps_gram", bufs=4, space="PSUM") as ps_g,
        tc.tile_pool(name="gwork", bufs=2) as gw,
    ):
        # ---- per-core support shard: 180-col window of ES at 135*pid ----
        ESsh = gw.tile([P, KT, XR], BF16)
        ASLsh = gw.tile([P, XR], BF16)
        pid = nc.vector.partition_id()
        w0 = pid * (3 * T)
        src = bass.AP(ES.tensor, ES[:].offset + w0,
                      [ES[:].ap[0], [SPAD, KT], [1, XR]],
                      dep_tracking_offset=ES[:].offset)
        nc.vector.tensor_copy(out=ESsh[:], in_=src)
        srcl = bass.AP(ASL.tensor, ASL[:].offset + w0,
                       [ASL[:].ap[0], [1, XR]],
                       dep_tracking_offset=ASL[:].offset)
        nc.vector.tensor_copy(out=ASLsh[:], in_=srcl)

        # ---- s-s gram for the shard; ship shard, gather full ss2 ----
        chunks = [(0, P), (P, XR)]
        for c0, c1 in chunks:
            sst = gw.tile([P, SPAD], BF16, tag="sst")
            for lo, hi in NSPLITS:
                pg = ps_g.tile([P, 384], F32, tag="pg")
                for k in range(KT):
                    nc.tensor.matmul(
                        out=pg[:c1 - c0, :], lhsT=ESsh[:, k, c0:c1],
                        rhs=ES[:, k, lo:hi], start=(k == 0), stop=False)
                nc.tensor.matmul(
                    out=pg[:c1 - c0, :], lhsT=ASLsh[:, c0:c1],
                    rhs=ASR[:, lo:hi], start=False, stop=True)
                nc.scalar.activation(
                    out=sst[:c1 - c0, lo:hi], in_=pg[:c1 - c0, :],
                    func=mybir.ActivationFunctionType.Relu, scale=-2.0)
            nc.sync.dma_start(out=ci_ss[c0:c1, :], in_=sst[:c1 - c0, :])
        nc.gpsimd.collective_compute(
            "AllGather", mybir.AluOpType.bypass,
            replica_groups=[list(range(N_CORES))],
            ins=[ci_ss[:]], outs=[ss_all[:]])
        if bisect == 25:
            dbg = pp.tile([16, 16], F32)
            ssd = pp.tile([16, 16], BF16)
            nc.sync.dma_start(out=ssd[:], in_=ss_all[:16, :16])
            nc.vector.tensor_copy(dbg[:], ssd[:])
            nc.sync.dma_start(out=out_v[:], in_=dbg[:])
            return

        # ---- q-s gram -> D2 ----
        for mq in range(QT):
            for lo, hi in NSPLITS:
                pg = ps_g.tile([P, 384], F32, tag="pg")
                for k in range(KT):
                    nc.tensor.matmul(
                        out=pg[:], lhsT=EQ[:, k, mq * P:(mq + 1) * P],
                        rhs=ES[:, k, lo:hi], start=(k == 0), stop=False)
                nc.tensor.matmul(
                    out=pg[:], lhsT=AQL[:, mq * P:(mq + 1) * P],
                    rhs=ASR[:, lo:hi], start=False, stop=True)
                nc.scalar.activation(
                    out=D2[:, mq, lo:hi], in_=pg[:],
                    func=mybir.ActivationFunctionType.Relu, scale=-2.0)


def _phase3(nc, tc, pp, D2, sb_invb, sb_gmat, ss_all, rec_in, rec_out,
            out_v, out_ms, bisect=0):
    with (
        tc.tile_pool(name="red", bufs=1) as rp,
        tc.tile_pool(name="gath", bufs=3) as gp,
        tc.tile_pool(name="mbp", bufs=2) as mbp,
        tc.tile_pool(name="cmpp", bufs=2) as cp,
        tc.tile_pool(name="ps3", bufs=1, space="PSUM") as ps3,
    ):
        V = rp.tile([P, QT, 16], F32)
        nc.vector.memset(V[:], 0.0)
        ave2 = rp.tile([P, QT, WAY], F32)
        idx = rp.tile([P, QT, WAY], U32)
        m8 = rp.tile([P, 8], BF16)
        i8 = rp.tile([P, 8], U32)
        dcall = rp.tile([P, QT, WAY, SHOT], F32)

        for c in range(WAY):
            for mq in range(QT):
                blk = D2[:, mq, c * CB:(c + 1) * CB]
                nc.vector.max(m8[:], blk)
                nc.vector.max_index(i8[:], m8[:], blk)
                nc.vector.tensor_copy(idx[:, mq, c:c + 1], i8[:, 0:1])
                nc.vector.tensor_copy(V[:, mq, c:c + 1], m8[:, 0:1])
                g = blk.rearrange("p (i j) -> p j i", j=SHOT)
                nc.vector.reduce_max(dcall[:, mq, c, :], g,
                                     axis=mybir.AxisListType.X)
        # ave2 = (mean_j sqrt(dcall))^2 + invalid-row bias
        dflat = dcall[:].rearrange("p a b c -> p (a b c)")
        nc.scalar.sqrt(out=dflat, in_=dflat)
        nc.vector.reduce_sum(
            ave2[:].rearrange("p a b -> p (a b)"),
            dcall[:].rearrange("p a b c -> p (a b) c"),
            axis=mybir.AxisListType.X)
        for mq in range(QT):
            nc.scalar.activation(
                out=ave2[:, mq, :], in_=ave2[:, mq, :],
                func=mybir.ActivationFunctionType.Square,
                bias=sb_invb[:, mq:mq + 1], scale=1.0 / SHOT)
        if bisect == 3:
            dbg = rp.tile([16, 16], F32)
            nc.vector.memset(dbg[:], 0.0)
            nc.vector.tensor_copy(dbg[:, :WAY], ave2[:16, 0, :])
            nc.sync.dma_start(out=out_v[:], in_=dbg[:])
            return

        # ---- global row remap: i -> 45*s + 45*g + t  with s = 5c + i//45,
        # g = min(s//3, 7), t = i%45.  No int div/mod on DVE, so do it in f32
        # with is_ge staircases (all values are small exact integers).
        rowi = rp.tile([P, QT, WAY], U32)
        cbase = rp.tile([P, QT, WAY], F32)
        for c in range(WAY):
            nc.gpsimd.memset(cbase[:, :, c:c + 1], float(5 * c))
        idxf = rp.tile([P, QT, WAY], F32)
        nc.vector.tensor_copy(out=idxf[:], in_=idx[:])
        s5 = rp.tile([P, QT, WAY], F32)
        nc.vector.tensor_scalar(out=s5[:], in0=idxf[:], scalar1=45.0,
                                scalar2=None, op0=mybir.AluOpType.is_ge)
        for k in (90.0, 135.0, 180.0):
            nc.vector.scalar_tensor_tensor(
                out=s5[:], in0=idxf[:], scalar=k, in1=s5[:],
                op0=mybir.AluOpType.is_ge, op1=mybir.AluOpType.add)
        tf = rp.tile([P, QT, WAY], F32)
        # t = i - 45*s5
        nc.vector.scalar_tensor_tensor(
            out=tf[:], in0=s5[:], scalar=-45.0, in1=idxf[:],
            op0=mybir.AluOpType.mult, op1=mybir.AluOpType.add)
        nc.vector.tensor_add(out=s5[:], in0=s5[:], in1=cbase[:])  # s = s5+5c
        gg = rp.tile([P, QT, WAY], F32)
        nc.vector.tensor_scalar(out=gg[:], in0=s5[:], scalar1=3.0,
                                scalar2=None, op0=mybir.AluOpType.is_ge)
        for k in (6.0, 9.0, 12.0, 15.0, 18.0, 21.0):
            nc.vector.scalar_tensor_tensor(
                out=gg[:], in0=s5[:], scalar=k, in1=gg[:],
                op0=mybir.AluOpType.is_ge, op1=mybir.AluOpType.add)
        # rowf = 45*s + 45*g + t
        nc.vector.scalar_tensor_tensor(
            out=tf[:], in0=s5[:], scalar=45.0, in1=tf[:],
            op0=mybir.AluOpType.mult, op1=mybir.AluOpType.add)
        nc.vector.scalar_tensor_tensor(
            out=tf[:], in0=gg[:], scalar=45.0, in1=tf[:],
            op0=mybir.AluOpType.mult, op1=mybir.AluOpType.add)
        nc.vector.tensor_copy(out=rowi[:], in_=tf[:])

        # ---- gather ss2 rows, compare vs ave2, accumulate record counts ----
        oh = rp.tile([P, WAY, WAY], BF16)
        nc.gpsimd.memset(oh[:], 0.0)
        for c in range(WAY):
            nc.gpsimd.memset(oh[:, c, c:c + 1], 1.0)
        rec5 = [ps3.tile([WAY, 384], F32, tag=f"rec{j}", name=f"rec{j}")
                for j in range(3)]
        for c in range(WAY):
            cmp = cp.tile([P, QT, SPAD], BF16, tag="cmp")
            for mq in range(QT):
                ssg = gp.tile([P, SPAD], BF16, tag="ssg")
                nc.gpsimd.indirect_dma_start(
                    out=ssg[:], out_offset=None,
                    in_=ss_all[:],
                    in_offset=bass.IndirectOffsetOnAxis(
                        ap=rowi[:, mq, c:c + 1], axis=0))
                nc.vector.tensor_scalar(
                    out=cmp[:, mq, :], in0=ssg[:],
                    scalar1=ave2[:, mq, c:c + 1], scalar2=None,
                    op0=mybir.AluOpType.is_gt)
                nc.gpsimd.memset(cmp[:, mq, c * CB:(c + 1) * CB], 0.0)
            for j, (lo, hi) in enumerate(NSPLITS):
                for mq in range(QT):
                    nc.tensor.matmul(
                        out=rec5[j][:], lhsT=oh[:, c, :], rhs=cmp[:, mq, lo:hi],
                        start=(c == 0 and mq == 0), stop=(c == WAY - 1 and mq == QT - 1))
        rec_sb = rp.tile([WAY, SPAD], F32)
        for j, (lo, hi) in enumerate(NSPLITS):
            nc.scalar.copy(out=rec_sb[:, lo:hi], in_=rec5[j][:])
        nc.sync.dma_start(out=rec_in[:], in_=rec_sb[:])
        if bisect == 4:
            dbg = rp.tile([16, 16], F32)
            nc.vector.memset(dbg[:], 0.0)
            nc.vector.tensor_copy(dbg[:WAY, :], rec_sb[:WAY, :16])
            nc.sync.dma_start(out=out_v[:], in_=dbg[:])
            return
        nc.gpsimd.collective_compute(
            "AllReduce", mybir.AluOpType.add,
            replica_groups=[list(range(N_CORES))],
            ins=[rec_in[:]], outs=[rec_out[:]])

        # overlap with the collective: sqrt distances for the masked means
        Ds = rp.tile([P, QT, SPAD], BF16)
        for mq in range(QT):
            nc.scalar.sqrt(out=Ds[:, mq, :], in_=D2[:, mq, :])

        recg = rp.tile([WAY, SPAD], F32)
        nc.sync.dma_start(out=recg[:], in_=rec_out[:])
        if bisect == 5:
            dbg = rp.tile([16, 16], F32)
            nc.vector.memset(dbg[:], 0.0)
            nc.vector.tensor_copy(dbg[:WAY, :], recg[:WAY, :16])
            nc.sync.dma_start(out=out_v[:], in_=dbg[:])
            return

        # thr per (class, block); mask; msum
        rsum = rp.tile([WAY, WAY], F32)
        nzc = rp.tile([WAY, SPAD], F32)
        nz = rp.tile([WAY, WAY], F32)
        thr = rp.tile([WAY, WAY], F32)
        gr = recg[:, :SROWS].rearrange("p (b s) -> p b s", b=WAY)
        nc.vector.reduce_sum(rsum[:], gr, axis=mybir.AxisListType.X)
        nc.vector.tensor_scalar(
            out=nzc[:, :SROWS], in0=recg[:, :SROWS],
            scalar1=0.5, scalar2=None, op0=mybir.AluOpType.is_gt)
        nc.vector.reduce_sum(
            nz[:], nzc[:, :SROWS].rearrange("p (b s) -> p b s", b=WAY),
            axis=mybir.AxisListType.X)
        nc.vector.tensor_scalar(
            out=nz[:], in0=nz[:], scalar1=1.0, scalar2=None,
            op0=mybir.AluOpType.max)
        nc.vector.reciprocal(out=nz[:], in_=nz[:])
        nc.vector.tensor_mul(out=thr[:], in0=rsum[:], in1=nz[:])

        mask = rp.tile([WAY, SPAD], BF16)
        nc.gpsimd.memset(mask[:, SROWS:], 0.0)
        for b in range(WAY):
            nc.vector.tensor_scalar(
                out=mask[:, b * CB:(b + 1) * CB],
                in0=recg[:, b * CB:(b + 1) * CB],
                scalar1=thr[:, b:b + 1], scalar2=None,
                op0=mybir.AluOpType.is_lt)
        msum = rp.tile([WAY, 1], F32)
        mskf = rp.tile([WAY, SROWS], F32)
        nc.vector.tensor_copy(out=mskf[:], in_=mask[:, :SROWS])
        nc.vector.reduce_sum(msum[:], mskf[:], axis=mybir.AxisListType.X)
        nc.sync.dma_start(out=out_ms[:], in_=msum[:])
        if bisect == 6:
            dbg = rp.tile([16, 16], F32)
            nc.vector.memset(dbg[:], 0.0)
            nc.vector.tensor_copy(dbg[:WAY, :], mask[:WAY, :16])
            nc.sync.dma_start(out=out_v[:], in_=dbg[:])
            return

        # masked mean distances: fused multiply + row-sum per (class, mq)
        scr = mbp.tile([P, 2, SPAD], BF16, tag="scr", bufs=1)
        for c in range(WAY):
            mrow = mbp.tile([1, SPAD], BF16, tag="mrow")
            nc.sync.dma_start(out=mrow[:], in_=mask[c:c + 1, :])
            mb = mbp.tile([P, SPAD], BF16, tag="mb")
            nc.gpsimd.partition_broadcast(mb[:], mrow[:])
            for mq in range(QT):
                nc.vector.scalar_tensor_tensor(
                    out=scr[:, (c * QT + mq) % 2, :SROWS],
                    in0=Ds[:, mq, :SROWS], scalar=1.0, in1=mb[:, :SROWS],
                    op0=mybir.AluOpType.mult, op1=mybir.AluOpType.mult,
                    accum_out=V[:, mq, WAY + c:WAY + c + 1])

        # sqrt the per-class maxima (cols 0..4 of V)
        for mq in range(QT):
            nc.scalar.sqrt(out=V[:, mq, :WAY], in_=V[:, mq, :WAY])
        if bisect == 7:
            dbg = rp.tile([16, 16], F32)
            nc.vector.tensor_copy(dbg[:], V[:16, 0, :])
            nc.sync.dma_start(out=out_v[:], in_=dbg[:])
            return

        # group rows by query: out[q, col] = sum_r gmat[r, q] * V[r, col]
        pv = ps3.tile([16, 16], F32, tag="pv")
        for t in range(QT):
            nc.tensor.matmul(
                out=pv[:], lhsT=sb_gmat[:, t * 16:(t + 1) * 16], rhs=V[:, t, :],
                start=(t == 0), stop=(t == QT - 1))
        vsb = rp.tile([16, 16], F32)
        nc.scalar.copy(out=vsb[:], in_=pv[:])
        nc.sync.dma_start(out=out_v[:], in_=vsb[:])


_CACHE = {}


def _get_nc():
    if "nc" not in _CACHE:
        _CACHE["nc"] = _build()
    return _CACHE["nc"]


def kernel(support_set, support_labels, queries, clsW_w, clsW_b):
    support_set = np.asarray(support_set, dtype=np.float32)
    queries = np.asarray(queries, dtype=np.float32)
    W = np.asarray(clsW_w, dtype=np.float32)
    b = np.asarray(clsW_b, dtype=np.float32)
    bf = ml_dtypes.bfloat16

    # weight tiles: wt[m, kp, h, ko, mf] = W[m*128+mf, h*2048+ko*128+kp]
    wt = np.ascontiguousarray(
        W.reshape(KT, P, 2, KT, P).transpose(0, 4, 2, 3, 1)).astype(bf)
    bias_h = np.ascontiguousarray(b.reshape(KT, P).T)

    # per-core frame matrices
    sup_frames = support_set.reshape(FS, D)
    in_maps = []
    rows = np.arange(QPAD)
    gm = np.zeros((QPAD, 16), np.float32)
    valid_all = rows < QROWS
    gm[valid_all, (rows // T)[valid_all].clip(0, 15)] = 1.0
    gmat_h = np.ascontiguousarray(gm.reshape(QT, P, 16).transpose(1, 0, 2))
    for k in range(N_CORES):
        qids = [(10 * k + j) % NQ for j in range(NQL)]
        qf = queries[qids].reshape(FQ, D)
        frames = np.concatenate([qf, sup_frames], axis=0)  # [350, 2048]
        xth = np.ascontiguousarray(
            frames.T.reshape(KT, P, F).transpose(1, 0, 2)).astype(bf)
        inv = np.where(rows < (QROWS if k < N_CORES - 1 else 5 * T), 0.0, BIG)
        inv = inv.astype(np.float32)
        invb_h = np.ascontiguousarray(inv.reshape(QT, P).T)
        in_maps.append({
            "wt": wt, "xt": xth, "bias": bias_h,
            "invb": invb_h, "gmat": gmat_h,
        })

    nc = _get_nc()
    _CACHE["last_in_maps"] = in_maps
    res = run_bass_kernel_spmd(nc, in_maps, list(range(N_CORES))).results

    msum = np.maximum(res[0]["out_ms"].reshape(WAY), 1.0)
    dist_max = np.zeros((NQ, WAY), np.float32)
    md_raw = np.zeros((NQ, WAY), np.float32)
    for k in range(N_CORES):
        v = res[k]["out_v"]
        for j in range(NQL):
            q = 10 * k + j
            if q >= NQ:
                break
            dist_max[q] = v[j, :WAY] / T
            md_raw[q] = v[j, WAY:2 * WAY]
    contrast = md_raw / (T * (WAY - 1) * msum[None, :])
    logits = dist_max / (contrast + dist_max)
    return dist_max, logits


# revision 26
# speedup vs baseline: 1.7116x; 1.4385x over previous
import sys

sys.path.insert(0, "/opt/trn_rl_repo")

import numpy as np
import ml_dtypes

import concourse.bass as bass
import concourse.tile as tile
from concourse import bacc, mybir
from concourse.bass_utils import run_bass_kernel_spmd

BF16 = mybir.dt.bfloat16
F32 = mybir.dt.float32
U32 = mybir.dt.uint32

N_CORES = 8
P = 128
WAY, SHOT, SEQ, TSS = 5, 5, 10, 2
T = 45                      # C(10,2) tuple pairs
NQ = 75                     # global queries
NQL = 10                    # queries per core (core 7: 5 real + 5 dup)
QROWS = NQL * T             # 450
QPAD = 512                  # 4 x 128
QT = 4                      # qrow tiles
SROWS = WAY * SHOT * T      # 1125
SPAD = 1152                 # 9 x 128
CB = SHOT * T               # 225 support cols per class
D = 2048                    # out dim
KT = 16                     # feat tiles (128 each)
FQ = NQL * SEQ              # 100 query frames
FS = WAY * SHOT * SEQ       # 250 support frames
F = FQ + FS                 # 350
NSPLITS = [(0, 384), (384, 768), (768, 1152)]
BIG = 1.0e18
XR = 141                    # s-s gram shard rows per core (ceil(1125/8))

# run lengths of PAIRS grouped by first index i: i pairs with j=i+1..9
RUNS = [(i, 9 - i) for i in range(SEQ - 1)]
TSTART = [0]
for _, r in RUNS[:-1]:
    TSTART.append(TSTART[-1] + r)


def _build():
    nc = bacc.Bacc(None, num_devices=N_CORES)

    # ---- I/O ----
    wt = nc.declare_dram_parameter("wt", [KT, P, 2 * KT * P], BF16, isOutput=False)
    xt = nc.declare_dram_parameter("xt", [P, KT * F], BF16, isOutput=False)
    bias = nc.declare_dram_parameter("bias", [P, KT], F32, isOutput=False)
    invb = nc.declare_dram_parameter("invb", [P, QT], F32, isOutput=False)
    gmat = nc.declare_dram_parameter("gmat", [P, QT * 16], F32, isOutput=False)
    ident = nc.declare_dram_parameter("ident", [P, P], BF16, isOutput=False)
    out_v = nc.declare_dram_parameter("out_v", [16, 16], F32, isOutput=True)
    out_ms = nc.declare_dram_parameter("out_ms", [WAY, 1], F32, isOutput=True)

    # internal DRAM
    ci_ss = nc.dram_tensor("ci_ss", [XR, SPAD], BF16)
    ss_all = nc.dram_tensor("ss_all", [N_CORES * XR, SPAD], BF16, addr_space="Shared")
    rec_in = nc.dram_tensor("rec_in", [WAY, SPAD], F32)
    rec_out = nc.dram_tensor("rec_out", [WAY, SPAD], F32, addr_space="Shared")

    import os
    bisect = int(os.environ.get("KBISECT", "0"))

    with tile.TileContext(nc) as tc:
        with tc.tile_pool(name="persist", bufs=1) as pp:
            sb_bias = pp.tile([P, KT], F32)
            nc.sync.dma_start(out=sb_bias[:], in_=bias[:])
            sb_invb = pp.tile([P, QT], F32)
            nc.sync.dma_start(out=sb_invb[:], in_=invb[:])
            sb_gmat = pp.tile([P, QT * 16], F32)
            nc.sync.dma_start(out=sb_gmat[:], in_=gmat[:])
            sb_id = pp.tile([P, P], BF16)
            nc.sync.dma_start(out=sb_id[:], in_=ident[:])

            EQ = pp.tile([P, KT, QPAD], BF16)    # relu'd query tuple rows
            ES = pp.tile([P, KT, SPAD], BF16)    # relu'd support tuple rows
            AQL = pp.tile([P, QPAD], BF16)       # aug lhsT for queries
            ASL = pp.tile([P, SPAD], BF16)       # aug lhsT for supports
            ASR = pp.tile([P, SPAD], BF16)       # aug rhs for supports
            D2 = pp.tile([P, QT, SPAD], BF16)    # squared q-s distances
            ones_c = pp.tile([P, 1], BF16)
            nc.vector.memset(ones_c[:], 1.0)

            _phase1(nc, tc, pp, xt, sb_bias, wt, EQ, ES,
                    AQL, ASL, ASR, ones_c)
            if bisect == 1:
                dbg = pp.tile([16, 16], F32)
                nc.vector.tensor_copy(dbg[:], EQ[:16, 0, :16])
                nc.sync.dma_start(out=out_v[:], in_=dbg[:])
            else:
                _phase2(nc, tc, pp, EQ, ES, AQL, ASL, ASR, D2, ci_ss, ss_all,
                        bisect, out_v)
                if bisect == 2:
                    dbg = pp.tile([16, 16], F32)
                    nc.vector.tensor_copy(dbg[:], D2[:16, 0, :16])
                    nc.sync.dma_start(out=out_v[:], in_=dbg[:])
                elif bisect != 25:
                    _phase3(nc, tc, pp, D2, sb_invb, sb_gmat, sb_id, ss_all,
                            rec_in, rec_out, out_v, out_ms, bisect)

    nc.compile()
    return nc


def _phase1(nc, tc, pp, xt, sb_bias, wt, EQ, ES, AQL, ASL, ASR,
            ones_c):
    # zero pad columns and aug tiles up front
    nc.vector.memset(EQ[:, :, QROWS:], 0.0)
    nc.gpsimd.memset(ES[:, :, SROWS:], 0.0)
    nc.vector.memset(AQL[:], 0.0)
    nc.gpsimd.memset(ASL[:], 0.0)
    nc.vector.memset(ASR[:], 0.0)

    with (
        tc.tile_pool(name="ph1", bufs=1) as p1,
        tc.tile_pool(name="wpool", bufs=2) as wp,
        tc.tile_pool(name="sqp", bufs=2) as sqp,
        tc.tile_pool(name="ps_ab", bufs=2, space="PSUM") as ps_ab,
        tc.tile_pool(name="ps_norm", bufs=1, space="PSUM") as ps_n,
    ):
        sb_xt = p1.tile([P, KT * F], BF16)
        nc.sync.dma_start(out=sb_xt[:], in_=xt[:])
        EA = p1.tile([P, KT, F], BF16)       # A-half embed + bias, per frame
        EB = p1.tile([P, KT, F], BF16)       # B-half embed, per frame
        qn = ps_n.tile([1, QPAD], F32, tag="qn")
        sn = [ps_n.tile([1, 384], F32, tag=f"sn{j}", name=f"sn{j}")
              for j in range(3)]

        # per-m software pipeline: 32 matmuls hide the DVE expansion, the
        # ACT evictions/relus/squares, and the norm-row matmuls of tile m-1
        for m in range(KT):
            wm = wp.tile([P, 2 * KT * P], BF16, tag="wm")
            nc.gpsimd.dma_start(out=wm[:], in_=wt[m])
            psA = ps_ab.tile([P, F], F32, tag="psA")
            psB = ps_ab.tile([P, F], F32, tag="psB")
            for k in range(KT):
                nc.tensor.matmul(
                    out=psA[:], lhsT=wm[:, k * P:(k + 1) * P],
                    rhs=sb_xt[:, k * F:(k + 1) * F],
                    start=(k == 0), stop=(k == KT - 1))
            for k in range(KT):
                nc.tensor.matmul(
                    out=psB[:], lhsT=wm[:, (KT + k) * P:(KT + k + 1) * P],
                    rhs=sb_xt[:, k * F:(k + 1) * F],
                    start=(k == 0), stop=(k == KT - 1))
            # evict with dtype cast; bias folded into A once per pair
            nc.scalar.activation(
                out=EA[:, m, :], in_=psA[:],
                func=mybir.ActivationFunctionType.Identity,
                bias=sb_bias[:, m:m + 1])
            nc.scalar.copy(out=EB[:, m, :], in_=psB[:])

            # tuple expansion for this m.  Query rows are laid out t*10+n
            # (pair-major) so out/in0/in1 all have contiguous last dims;
            # support rows must stay class-blocked n*45+t (contiguous tt).
            EQv = EQ[:, m, :QROWS].rearrange("p (t n) -> p t n", n=NQL)
            ESv = ES[:, m, :SROWS].rearrange("p (n t) -> p n t", t=T)
            EAs = EA[:, m, FQ:].rearrange("p (n s) -> p n s", s=SEQ)
            EBs = EB[:, m, FQ:].rearrange("p (n s) -> p n s", s=SEQ)
            for (i, run), ts0 in zip(RUNS, TSTART):
                nc.vector.tensor_add(
                    out=EQv[:, ts0:ts0 + run, :],
                    in0=EA[:, m, i * NQL:(i + 1) * NQL].unsqueeze(1)
                        .broadcast_to((P, run, NQL)),
                    in1=EB[:, m, (i + 1) * NQL:(i + 1 + run) * NQL]
                        .rearrange("p (t n) -> p t n", n=NQL))
                nc.vector.tensor_add(
                    out=ESv[:, :, ts0:ts0 + run],
                    in0=EAs[:, :, i:i + 1].broadcast_to((P, WAY * SHOT, run)),
                    in1=EBs[:, :, i + 1:i + 1 + run])
            # relu in place + squares on the scalar engine
            nc.scalar.activation(
                out=ES[:, m, :], in_=ES[:, m, :],
                func=mybir.ActivationFunctionType.Relu)
            nc.scalar.activation(
                out=EQ[:, m, :], in_=EQ[:, m, :],
                func=mybir.ActivationFunctionType.Relu)
            sq = sqp.tile([P, SPAD], BF16, tag="sq")
            nc.scalar.square(out=sq[:], in_=ES[:, m, :])
            sqq = sqp.tile([P, QPAD], BF16, tag="sqq")
            nc.scalar.square(out=sqq[:], in_=EQ[:, m, :])
            for j, (lo, hi) in enumerate(NSPLITS):
                nc.tensor.matmul(
                    out=sn[j][:], lhsT=ones_c[:], rhs=sq[:, lo:hi],
                    start=(m == 0), stop=(m == KT - 1))
            nc.tensor.matmul(
                out=qn[:], lhsT=ones_c[:], rhs=sqq[:],
                start=(m == 0), stop=(m == KT - 1))

        # ---- aug tiles (bf16x2 error-compensated norms) ----
        # ASR: p0=1, p1=1, p2=sn_hi, p3=sn_lo ; ASL: p0=-sn_hi/2, p1=-sn_lo/2,
        # p2=p3=-1/2 ; AQL: p0=-qn_hi/2, p1=-qn_lo/2, p2=p3=-1/2
        # compute-engine APs must start at partition 0, so rows for
        # partitions >= 1 are staged at p0 and DMA'd into place
        snf = p1.tile([1, SPAD], F32)
        for j, (lo, hi) in enumerate(NSPLITS):
            nc.scalar.copy(out=snf[:, lo:hi], in_=sn[j][:])
        snh_bf = p1.tile([1, SPAD], BF16)
        nc.scalar.copy(out=snh_bf[:], in_=snf[:])              # sn_hi
        snh = p1.tile([1, SPAD], F32)
        nc.scalar.copy(out=snh[:], in_=snh_bf[:])
        snl = p1.tile([1, SPAD], F32)
        nc.vector.tensor_sub(out=snl[:], in0=snf[:], in1=snh[:])
        snl_bf = p1.tile([1, SPAD], BF16)
        nc.vector.tensor_copy(out=snl_bf[:], in_=snl[:])       # sn_lo
        ashi = p1.tile([1, SPAD], BF16)
        nc.scalar.mul(out=ashi[:], in_=snh_bf[:], mul=-0.5)
        aslo = p1.tile([1, SPAD], BF16)
        nc.scalar.mul(out=aslo[:], in_=snl_bf[:], mul=-0.5)
        nc.vector.memset(ASR[0:2, :], 1.0)
        nc.sync.dma_start(out=ASR[2:3, :], in_=snh_bf[:])
        nc.sync.dma_start(out=ASR[3:4, :], in_=snl_bf[:])
        nc.vector.memset(ASL[0:4, :], -0.5)
        nc.sync.dma_start(out=ASL[0:1, :], in_=ashi[:])
        nc.sync.dma_start(out=ASL[1:2, :], in_=aslo[:])

        qnf = p1.tile([1, QPAD], F32)
        nc.scalar.copy(out=qnf[:], in_=qn[:])
        qh = p1.tile([1, QPAD], BF16)
        nc.scalar.mul(out=qh[:], in_=qnf[:], mul=-0.5)         # -qn/2 hi (bf16)
        qhf = p1.tile([1, QPAD], F32)
        nc.scalar.copy(out=qhf[:], in_=qh[:])
        qlf = p1.tile([1, QPAD], F32)
        # residual: (-qn/2) - hi
        nc.vector.scalar_tensor_tensor(
            out=qlf[:], in0=qnf[:], scalar=-0.5, in1=qhf[:],
            op0=mybir.AluOpType.mult, op1=mybir.AluOpType.subtract)
        ql = p1.tile([1, QPAD], BF16)
        nc.vector.tensor_copy(out=ql[:], in_=qlf[:])
        nc.vector.memset(AQL[0:4, :], -0.5)
        nc.sync.dma_start(out=AQL[0:1, :], in_=qh[:])
        nc.sync.dma_start(out=AQL[1:2, :], in_=ql[:])


def _phase2(nc, tc, pp, EQ, ES, AQL, ASL, ASR, D2, ci_ss, ss_all, bisect,
            out_v):
    with (
        tc.tile_pool(name="ps_gram", bufs=4, space="PSUM") as ps_g,
        tc.tile_pool(name="gwork", bufs=2) as gw,
    ):
        # ---- per-core support shard: 180-col window of ES at 135*pid ----
        ESsh = gw.tile([P, KT, XR], BF16)
        ASLsh = gw.tile([P, XR], BF16)
        pid = nc.vector.partition_id()
        w0 = pid * XR
        src = bass.AP(ES.tensor, ES[:].offset + w0,
                      [ES[:].ap[0], [SPAD, KT], [1, XR]],
                      dep_tracking_offset=ES[:].offset)
        nc.vector.tensor_copy(out=ESsh[:], in_=src)
        srcl = bass.AP(ASL.tensor, ASL[:].offset + w0,
                       [ASL[:].ap[0], [1, XR]],
                       dep_tracking_offset=ASL[:].offset)
        nc.vector.tensor_copy(out=ASLsh[:], in_=srcl)

        # ---- s-s gram for the shard; ship shard, gather full ss2 ----
        chunks = [(0, P), (P, XR)]
        for c0, c1 in chunks:
            sst = gw.tile([P, SPAD], BF16, tag="sst")
            for lo, hi in NSPLITS:
                pg = ps_g.tile([P, 384], F32, tag="pg")
                for k in range(KT):
                    nc.tensor.matmul(
                        out=pg[:c1 - c0, :], lhsT=ESsh[:, k, c0:c1],
                        rhs=ES[:, k, lo:hi], start=(k == 0), stop=False)
                nc.tensor.matmul(
                    out=pg[:c1 - c0, :], lhsT=ASLsh[:, c0:c1],
                    rhs=ASR[:, lo:hi], start=False, stop=True)
                nc.scalar.activation(
                    out=sst[:c1 - c0, lo:hi], in_=pg[:c1 - c0, :],
                    func=mybir.ActivationFunctionType.Relu, scale=-2.0)
            nc.sync.dma_start(out=ci_ss[c0:c1, :], in_=sst[:c1 - c0, :])
        nc.gpsimd.collective_compute(
            "AllGather", mybir.AluOpType.bypass,
            replica_groups=[list(range(N_CORES))],
            ins=[ci_ss[:]], outs=[ss_all[:]])
        if bisect == 25:
            dbg = pp.tile([16, 16], F32)
            ssd = pp.tile([16, 16], BF16)
            nc.sync.dma_start(out=ssd[:], in_=ss_all[:16, :16])
            nc.vector.tensor_copy(dbg[:], ssd[:])
            nc.sync.dma_start(out=out_v[:], in_=dbg[:])
            return

        # ---- q-s gram -> D2 ----
        for mq in range(QT):
            for lo, hi in NSPLITS:
                pg = ps_g.tile([P, 384], F32, tag="pg")
                for k in range(KT):
                    nc.tensor.matmul(
                        out=pg[:], lhsT=EQ[:, k, mq * P:(mq + 1) * P],
                        rhs=ES[:, k, lo:hi], start=(k == 0), stop=False)
                nc.tensor.matmul(
                    out=pg[:], lhsT=AQL[:, mq * P:(mq + 1) * P],
                    rhs=ASR[:, lo:hi], start=False, stop=True)
                nc.scalar.activation(
                    out=D2[:, mq, lo:hi], in_=pg[:],
                    func=mybir.ActivationFunctionType.Relu, scale=-2.0)


def _phase3(nc, tc, pp, D2, sb_invb, sb_gmat, sb_id, ss_all, rec_in, rec_out,
            out_v, out_ms, bisect=0):
    with (
        tc.tile_pool(name="red", bufs=1) as rp,
        tc.tile_pool(name="gath", bufs=3) as gp,
        tc.tile_pool(name="cmpp", bufs=2) as cp,
        tc.tile_pool(name="ps3", bufs=1, space="PSUM") as ps3,
    ):
        V = rp.tile([P, QT, 16], F32)
        nc.vector.memset(V[:], 0.0)
        ave2 = rp.tile([P, QT, WAY], F32)
        idx = rp.tile([P, QT, WAY], U32)
        m8 = rp.tile([P, 8], BF16)
        i8 = rp.tile([P, 8], U32)
        dcall = rp.tile([P, QT, WAY, SHOT], F32)

        for c in range(WAY):
            for mq in range(QT):
                blk = D2[:, mq, c * CB:(c + 1) * CB]
                nc.vector.max(m8[:], blk)
                nc.vector.max_index(i8[:], m8[:], blk)
                nc.vector.tensor_copy(idx[:, mq, c:c + 1], i8[:, 0:1])
                nc.vector.tensor_copy(V[:, mq, c:c + 1], m8[:, 0:1])
                g = blk.rearrange("p (i j) -> p j i", j=SHOT)
                nc.vector.reduce_max(dcall[:, mq, c, :], g,
                                     axis=mybir.AxisListType.X)
        # ave2 = (mean_j sqrt(dcall))^2 + invalid-row bias
        dflat = dcall[:].rearrange("p a b c -> p (a b c)")
        nc.scalar.sqrt(out=dflat, in_=dflat)
        nc.vector.reduce_sum(
            ave2[:].rearrange("p a b -> p (a b)"),
            dcall[:].rearrange("p a b c -> p (a b) c"),
            axis=mybir.AxisListType.X)
        for mq in range(QT):
            nc.scalar.activation(
                out=ave2[:, mq, :], in_=ave2[:, mq, :],
                func=mybir.ActivationFunctionType.Square,
                bias=sb_invb[:, mq:mq + 1], scale=1.0 / SHOT)
        if bisect == 3:
            dbg = rp.tile([16, 16], F32)
            nc.vector.memset(dbg[:], 0.0)
            nc.vector.tensor_copy(dbg[:, :WAY], ave2[:16, 0, :])
            nc.sync.dma_start(out=out_v[:], in_=dbg[:])
            return

        # ---- global ss_all row = 225*c + idx (141-row shards concatenate
        # to exactly the global class-blocked row order)
        rowi = rp.tile([P, QT, WAY], U32)
        cbase = rp.tile([P, QT, WAY], U32)
        for c in range(WAY):
            nc.gpsimd.memset(cbase[:, :, c:c + 1], CB * c)
        nc.vector.tensor_add(out=rowi[:], in0=idx[:], in1=cbase[:])

        # sqrt distances + transposed copies for the masked-mean matmuls
        # (independent of the record collective; runs while PE is idle)
        Ds = rp.tile([P, QT, SPAD], BF16)
        for mq in range(QT):
            nc.scalar.sqrt(out=Ds[:, mq, :], in_=D2[:, mq, :])
        DsT = rp.tile([P, SPAD // P, QPAD], BF16)
        with tc.tile_pool(name="ps_t", bufs=4, space="PSUM") as ps_t:
            for st in range(SPAD // P):
                for mq in range(QT):
                    pt = ps_t.tile([P, P], BF16, tag="pt")
                    nc.tensor.transpose(
                        out=pt[:], in_=Ds[:, mq, st * P:(st + 1) * P],
                        identity=sb_id[:])
                    nc.scalar.copy(out=DsT[:, st, mq * P:(mq + 1) * P],
                                   in_=pt[:])

        # ---- gather ss2 rows, compare vs ave2, accumulate record counts ----
        oh = rp.tile([P, WAY, WAY], BF16)
        nc.gpsimd.memset(oh[:], 0.0)
        for c in range(WAY):
            nc.gpsimd.memset(oh[:, c, c:c + 1], 1.0)
        rec5 = [ps3.tile([WAY, 384], F32, tag=f"rec{j}", name=f"rec{j}")
                for j in range(3)]
        for c in range(WAY):
            cmp = cp.tile([P, QT, SPAD], BF16, tag="cmp")
            for mq in range(QT):
                ssg = gp.tile([P, SPAD], BF16, tag="ssg")
                nc.gpsimd.indirect_dma_start(
                    out=ssg[:], out_offset=None,
                    in_=ss_all[:],
                    in_offset=bass.IndirectOffsetOnAxis(
                        ap=rowi[:, mq, c:c + 1], axis=0))
                nc.vector.tensor_scalar(
                    out=cmp[:, mq, :], in0=ssg[:],
                    scalar1=ave2[:, mq, c:c + 1], scalar2=None,
                    op0=mybir.AluOpType.is_gt)
                nc.gpsimd.memset(cmp[:, mq, c * CB:(c + 1) * CB], 0.0)
            for j, (lo, hi) in enumerate(NSPLITS):
                for mq in range(QT):
                    nc.tensor.matmul(
                        out=rec5[j][:], lhsT=oh[:, c, :], rhs=cmp[:, mq, lo:hi],
                        start=(c == 0 and mq == 0), stop=(c == WAY - 1 and mq == QT - 1))
        rec_sb = rp.tile([WAY, SPAD], F32)
        for j, (lo, hi) in enumerate(NSPLITS):
            nc.scalar.copy(out=rec_sb[:, lo:hi], in_=rec5[j][:])
        nc.sync.dma_start(out=rec_in[:], in_=rec_sb[:])
        if bisect == 4:
            dbg = rp.tile([16, 16], F32)
            nc.vector.memset(dbg[:], 0.0)
            nc.vector.tensor_copy(dbg[:WAY, :], rec_sb[:WAY, :16])
            nc.sync.dma_start(out=out_v[:], in_=dbg[:])
            return
        nc.gpsimd.collective_compute(
            "AllReduce", mybir.AluOpType.add,
            replica_groups=[list(range(N_CORES))],
            ins=[rec_in[:]], outs=[rec_out[:]])

        recg = rp.tile([WAY, SPAD], F32)
        nc.sync.dma_start(out=recg[:], in_=rec_out[:])
        if bisect == 5:
            dbg = rp.tile([16, 16], F32)
            nc.vector.memset(dbg[:], 0.0)
            nc.vector.tensor_copy(dbg[:WAY, :], recg[:WAY, :16])
            nc.sync.dma_start(out=out_v[:], in_=dbg[:])
            return

        # thr per (class, block); mask; msum
        rsum = rp.tile([WAY, WAY], F32)
        nzc = rp.tile([WAY, SPAD], F32)
        nz = rp.tile([WAY, WAY], F32)
        thr = rp.tile([WAY, WAY], F32)
        gr = recg[:, :SROWS].rearrange("p (b s) -> p b s", b=WAY)
        nc.vector.reduce_sum(rsum[:], gr, axis=mybir.AxisListType.X)
        nc.vector.tensor_scalar(
            out=nzc[:, :SROWS], in0=recg[:, :SROWS],
            scalar1=0.5, scalar2=None, op0=mybir.AluOpType.is_gt)
        nc.vector.reduce_sum(
            nz[:], nzc[:, :SROWS].rearrange("p (b s) -> p b s", b=WAY),
            axis=mybir.AxisListType.X)
        nc.vector.tensor_scalar(
            out=nz[:], in0=nz[:], scalar1=1.0, scalar2=None,
            op0=mybir.AluOpType.max)
        nc.vector.reciprocal(out=nz[:], in_=nz[:])
        nc.vector.tensor_mul(out=thr[:], in0=rsum[:], in1=nz[:])

        mask = rp.tile([WAY, SPAD], BF16)
        nc.gpsimd.memset(mask[:, SROWS:], 0.0)
        for b in range(WAY):
            nc.vector.tensor_scalar(
                out=mask[:, b * CB:(b + 1) * CB],
                in0=recg[:, b * CB:(b + 1) * CB],
                scalar1=thr[:, b:b + 1], scalar2=None,
                op0=mybir.AluOpType.is_lt)
        msum = rp.tile([WAY, 1], F32)
        mskf = rp.tile([WAY, SROWS], F32)
        nc.vector.tensor_copy(out=mskf[:], in_=mask[:, :SROWS])
        nc.vector.reduce_sum(msum[:], mskf[:], axis=mybir.AxisListType.X)
        nc.sync.dma_start(out=out_ms[:], in_=msum[:])
        if bisect == 6:
            dbg = rp.tile([16, 16], F32)
            nc.vector.memset(dbg[:], 0.0)
            nc.vector.tensor_copy(dbg[:WAY, :], mask[:WAY, :16])
            nc.sync.dma_start(out=out_v[:], in_=dbg[:])
            return

        # masked mean distances as matmuls: V[q, 5+c] = sum_s2 DsT[s2,q]*maskT[s2,c]
        maskT = rp.tile([P, SPAD // P, WAY], BF16)
        with tc.tile_pool(name="ps_m", bufs=2, space="PSUM") as ps_m:
            for st in range(SPAD // P):
                pt = ps_m.tile([P, WAY], BF16, tag="ptm")
                nc.tensor.transpose(
                    out=pt[:, :WAY], in_=mask[:WAY, st * P:(st + 1) * P],
                    identity=sb_id[:WAY, :WAY])
                nc.scalar.copy(out=maskT[:, st, :], in_=pt[:, :WAY])
            for mq in range(QT):
                pv5 = ps_m.tile([P, WAY], F32, tag="pv5")
                for st in range(SPAD // P):
                    nc.tensor.matmul(
                        out=pv5[:], lhsT=DsT[:, st, mq * P:(mq + 1) * P],
                        rhs=maskT[:, st, :], start=(st == 0),
                        stop=(st == SPAD // P - 1))
                nc.scalar.copy(out=V[:, mq, WAY:2 * WAY], in_=pv5[:])

        # sqrt the per-class maxima (cols 0..4 of V)
        for mq in range(QT):
            nc.scalar.sqrt(out=V[:, mq, :WAY], in_=V[:, mq, :WAY])
        if bisect == 7:
            dbg = rp.tile([16, 16], F32)
            nc.vector.tensor_copy(dbg[:], V[:16, 0, :])
            nc.sync.dma_start(out=out_v[:], in_=dbg[:])
            return

        # group rows by query: out[q, col] = sum_r gmat[r, q] * V[r, col]
        pv = ps3.tile([16, 16], F32, tag="pv")
        for t in range(QT):
            nc.tensor.matmul(
                out=pv[:], lhsT=sb_gmat[:, t * 16:(t + 1) * 16], rhs=V[:, t, :],
                start=(t == 0), stop=(t == QT - 1))
        vsb = rp.tile([16, 16], F32)
        nc.scalar.copy(out=vsb[:], in_=pv[:])
        nc.sync.dma_start(out=out_v[:], in_=vsb[:])


_CACHE = {}


def _get_nc():
    if "nc" not in _CACHE:
        _CACHE["nc"] = _build()
    return _CACHE["nc"]


def kernel(support_set, support_labels, queries, clsW_w, clsW_b):
    support_set = np.asarray(support_set, dtype=np.float32)
    queries = np.asarray(queries, dtype=np.float32)
    W = np.asarray(clsW_w, dtype=np.float32)
    b = np.asarray(clsW_b, dtype=np.float32)
    bf = ml_dtypes.bfloat16

    # weight tiles: wt[m, kp, h, ko, mf] = W[m*128+mf, h*2048+ko*128+kp]
    wt = np.ascontiguousarray(
        W.reshape(KT, P, 2, KT, P).transpose(0, 4, 2, 3, 1)).astype(bf)
    bias_h = np.ascontiguousarray(b.reshape(KT, P).T)

    # per-core frame matrices.  Query tuple rows are laid out t*10 + n
    # (pair-major) and query frames seq-major (seq*10 + n) so the kernel's
    # pair-expansion adds are fully contiguous.
    sup_frames = support_set.reshape(FS, D)
    in_maps = []
    rows = np.arange(QPAD)
    gm = np.zeros((QPAD, 16), np.float32)
    valid_all = rows < QROWS
    gm[valid_all, (rows % NQL)[valid_all]] = 1.0
    gmat_h = np.ascontiguousarray(gm.reshape(QT, P, 16).transpose(1, 0, 2))
    ident_h = np.eye(P, dtype=np.float32).astype(bf)
    for k in range(N_CORES):
        qids = [(10 * k + j) % NQ for j in range(NQL)]
        qf = queries[qids].transpose(1, 0, 2).reshape(FQ, D)  # seq-major
        frames = np.concatenate([qf, sup_frames], axis=0)  # [350, 2048]
        xth = np.ascontiguousarray(
            frames.T.reshape(KT, P, F).transpose(1, 0, 2)).astype(bf)
        valid = valid_all.copy()
        if k == N_CORES - 1:
            valid &= (rows % NQL) < 5
        inv = np.where(valid, 0.0, BIG).astype(np.float32)
        invb_h = np.ascontiguousarray(inv.reshape(QT, P).T)
        in_maps.append({
            "wt": wt, "xt": xth, "bias": bias_h,
            "invb": invb_h, "gmat": gmat_h, "ident": ident_h,
        })

    nc = _get_nc()
    _CACHE["last_in_maps"] = in_maps
    res = run_bass_kernel_spmd(nc, in_maps, list(range(N_CORES))).results

    msum = np.maximum(res[0]["out_ms"].reshape(WAY), 1.0)
    dist_max = np.zeros((NQ, WAY), np.float32)
    md_raw = np.zeros((NQ, WAY), np.float32)
    for k in range(N_CORES):
        v = res[k]["out_v"]
        for j in range(NQL):
            q = 10 * k + j
            if q >= NQ:
                break
            dist_max[q] = v[j, :WAY] / T
            md_raw[q] = v[j, WAY:2 * WAY]
    contrast = md_raw / (T * (WAY - 1) * msum[None, :])
    logits = dist_max / (contrast + dist_max)
    return dist_max, logits
